# revision 8
# baseline (speedup 1.0000x reference)
"""KDE log-density kernel for Trainium2, SPMD across 8 NeuronCores.

Computes log_p[m] = logsumexp_n(-scale * ||X[m] - svs[n]||^2) - log(N)
                    + (D/2) * log(scale/pi)

Sharding: X rows split across 8 cores (1024 rows each); svs + scale
replicated. logsumexp over N is local to each row, no collectives.

Per-core algorithm:
  - Pass over svs: s2[n] = sum_d svs[n,d]^2 (ACT Square+accum), cast
    2*svs to bf16 and bounce through DRAM for a DMA transpose into
    svsT [128(d), 4, 8192(n)] resident in SBUF.
  - Same for X: x2, bf16 X^T tiles.
  - s2 broadcast to all 128 partitions via identity-transpose + rank-1
    matmuls -> s2b [128, 8192] f32.
  - Main loop over 8 m-tiles x 16 n-chunks: 4 matmuls (K=128 each)
    accumulate psum = 2*X.svs; DVE tensor_tensor_reduce computes
    u = psum - s2[n] in-place and max_n(u) in one op; online-max
    rescale (flash style); ACT computes exp(scale*u + bias) in-place
    with a fused row-sum accum_out.
  - log_p = ln(acc) + M + C,  C = -ln(N) + (D/2)*(ln(scale) - ln(pi)).
"""

import sys

for _p in ("/opt/trn_rl_repo", "/opt/pypackages"):
    if _p not in sys.path:
        sys.path.insert(0, _p)

import numpy as np

M_FULL, N, D = 8192, 8192, 512
NCORES = 8
M_LOC = M_FULL // NCORES  # 1024
P = 128
KS = D // P               # 4 k-subtiles
MT = M_LOC // P           # 8 m-tiles per core
NCH = 512                 # n-chunk (one PSUM bank of f32)
NCHUNKS = N // NCH        # 16

_CACHE = {}
USE_ONLINE_MAX = False


def _build_nc():
    import concourse.mybir as mybir
    import concourse.tile as tile
    from concourse import bacc

    f32 = mybir.dt.float32
    bf16 = mybir.dt.bfloat16
    AF = mybir.ActivationFunctionType
    ALU = mybir.AluOpType
    AX = mybir.AxisListType

    nc = bacc.Bacc(None, target_bir_lowering=False, debug=True)
    Xd = nc.declare_dram_parameter("X", [M_LOC, D], f32, isOutput=False)
    Sd = nc.declare_dram_parameter("svs", [N, D], f32, isOutput=False)
    scd = nc.declare_dram_parameter("scale", [1], f32, isOutput=False)
    outd = nc.declare_dram_parameter("out", [M_LOC, 1], f32, isOutput=True)

    LOG_CONST = float(-np.log(N) - (D / 2) * np.log(np.pi))

    with tile.TileContext(nc) as tc:
        with (
            tc.tile_pool(name="const", bufs=1) as cp,
            tc.tile_pool(name="work", bufs=3) as wp,
            tc.tile_pool(name="small", bufs=4) as sp,
            tc.tile_pool(name="mmpsum", bufs=6, space="PSUM") as pp,
            tc.tile_pool(name="bcpsum", bufs=2, space="PSUM") as pb,
            tc.tile_pool(name="dram", bufs=1, space="DRAM") as dp,
        ):
            # ---------------- constants ----------------
            scale_bc = cp.tile([P, 1], f32)
            nc.sync.dma_start(scale_bc[:], scd[None, :].to_broadcast((P, 1)))
            negscale = cp.tile([P, 1], f32)
            nc.scalar.mul(negscale[:], scale_bc[:], -1.0)
            # C = (D/2)*ln(scale) + (-ln(N) - (D/2)*ln(pi))
            C_bc = cp.tile([P, 1], f32)
            nc.scalar.activation(C_bc[:], scale_bc[:], AF.Ln)
            nc.vector.tensor_scalar(
                C_bc[:], C_bc[:], float(D / 2), LOG_CONST, ALU.mult, ALU.add
            )

            # ---------------- X pass: x2 + bf16(X) ----------------
            x2sc = cp.tile([P, MT], f32)  # -scale * x2, per m-tile column
            x2_all = cp.tile([P, MT], f32)
            xb_dram = dp.tile([M_LOC, D], bf16)
            for t in range(MT):
                xv = wp.tile([P, D], f32, tag="in_f32")
                nc.sync.dma_start(xv[:], Xd[t * P:(t + 1) * P, :])
                xsq = wp.tile([P, D], f32, tag="sq")
                nc.scalar.activation(
                    xsq[:], xv[:], AF.Square, accum_out=x2_all[:, t:t + 1]
                )
                xbv = wp.tile([P, D], bf16, tag="out_b16")
                nc.vector.tensor_copy(xbv[:], xv[:])
                nc.sync.dma_start(xb_dram[t * P:(t + 1) * P, :], xbv[:])
                nc.vector.tensor_tensor(
                    x2sc[:, t:t + 1], x2_all[:, t:t + 1], negscale[:], ALU.mult
                )

            xT = cp.tile([P, KS, M_LOC], bf16)
            nc.sync.dma_start_transpose(xT[:], xb_dram[:])

            # ---------------- svs pass: s2 + bf16(2*svs) ----------------
            # contiguous chunks; 8 n-blocks of 1024 rows, transposed as
            # soon as their 8 chunks are stored so the main loop can start.
            NBLK = 8
            BROWS = N // NBLK  # 1024
            s2_dram = dp.tile([N], f32)
            s2b = cp.tile([P, N], f32)
            svs2b = [
                dp.tile([BROWS, D], bf16, tag=f"svs2b{b}", name=f"svs2b{b}")
                for b in range(NBLK)
            ]
            svsT = cp.tile([P, KS, N], bf16)   # [d%128, d//128, n]
            for b in range(NBLK):
                for cc in range(BROWS // P):
                    c = b * (BROWS // P) + cc
                    sv = wp.tile([P, D], f32, tag="in_f32")
                    nc.sync.dma_start(sv[:], Sd[c * P:(c + 1) * P, :])
                    sq = wp.tile([P, D], f32, tag="sq")
                    s2c = sp.tile([P, 1], f32, tag="s2c")
                    nc.scalar.activation(sq[:], sv[:], AF.Square, accum_out=s2c[:])
                    nc.sync.dma_start(s2_dram[c * P:(c + 1) * P, None], s2c[:])
                    svb = wp.tile([P, D], bf16, tag="out_b16")
                    nc.vector.tensor_scalar_mul(svb[:], sv[:], 2.0)
                    nc.sync.dma_start(svs2b[b][cc * P:(cc + 1) * P, :], svb[:])
                nc.sync.dma_start_transpose(
                    svsT[:, :, b * BROWS:(b + 1) * BROWS], svs2b[b][:]
                )
                nc.sync.dma_start(
                    s2b[:, b * BROWS:(b + 1) * BROWS],
                    s2_dram[None, b * BROWS:(b + 1) * BROWS].to_broadcast(
                        (P, BROWS)
                    ),
                )

            # ---------------- main loop ----------------
            if USE_ONLINE_MAX:
                for t in range(MT):
                    B_t = sp.tile([P, 1], f32, tag=f"B{t}")
                    acc_t = sp.tile([P, 1], f32, tag=f"acc{t}")
                    for j in range(NCHUNKS):
                        ps = pp.tile([P, NCH], f32, tag="mm")
                        for kt in range(KS):
                            nc.tensor.matmul(
                                ps[:],
                                xT[:, kt, t * P:(t + 1) * P],
                                svsT[:, kt, j * NCH:(j + 1) * NCH],
                                start=(kt == 0),
                                stop=(kt == KS - 1),
                            )
                        mxu = sp.tile([P, 1], f32, tag="mxu")
                        uu = wp.tile([P, NCH], f32, tag="uu")
                        nc.vector.tensor_tensor_reduce(
                            out=uu[:],
                            in0=ps[:],
                            in1=s2b[:, j * NCH:(j + 1) * NCH],
                            scale=1.0,
                            scalar=-3.4e38,
                            op0=ALU.subtract,
                            op1=ALU.max,
                            accum_out=mxu[:],
                        )
                        nm = sp.tile([P, 1], f32, tag="nm")
                        nc.vector.tensor_tensor(nm[:], mxu[:], negscale[:], ALU.mult)
                        pj = sp.tile([P, 1], f32, tag="pj")
                        ee = wp.tile([P, NCH], f32, tag="ee")
                        if j == 0:
                            nc.vector.tensor_copy(B_t[:], nm[:])
                            nc.scalar.activation(
                                ee[:], uu[:], AF.Exp,
                                bias=B_t[:], scale=scale_bc[:], accum_out=pj[:],
                            )
                            nc.vector.tensor_copy(acc_t[:], pj[:])
                        else:
                            dd = sp.tile([P, 1], f32, tag="dd")
                            nc.vector.tensor_scalar(
                                dd[:], nm[:], B_t[:], 0.0, ALU.subtract, ALU.min
                            )
                            nc.vector.tensor_tensor(B_t[:], B_t[:], dd[:], ALU.add)
                            rr = sp.tile([P, 1], f32, tag="rr")
                            nc.scalar.activation(rr[:], dd[:], AF.Exp)
                            nc.scalar.activation(
                                ee[:], uu[:], AF.Exp,
                                bias=B_t[:], scale=scale_bc[:], accum_out=pj[:],
                            )
                            nc.vector.tensor_scalar(
                                acc_t[:], acc_t[:], rr[:], pj[:], ALU.mult, ALU.add
                            )
                    mfin = sp.tile([P, 1], f32, tag="mfin")
                    nc.vector.tensor_tensor(
                        mfin[:], x2sc[:, t:t + 1], B_t[:], ALU.subtract
                    )
                    nc.vector.tensor_tensor(mfin[:], mfin[:], C_bc[:], ALU.add)
                    lp = sp.tile([P, 1], f32, tag="lp")
                    nc.scalar.activation(lp[:], acc_t[:], AF.Ln)
                    nc.vector.tensor_tensor(lp[:], lp[:], mfin[:], ALU.add)
                    nc.sync.dma_start(outd[t * P:(t + 1) * P, :], lp[:])
            else:
                parts = [
                    sp.tile([P, NCHUNKS], f32, tag=f"part{t}", name=f"part{t}")
                    for t in range(MT)
                ]
                for j in range(NCHUNKS):
                    for t in range(MT):
                        ps = pp.tile([P, NCH], f32, tag="mm")
                        for kt in range(KS):
                            nc.tensor.matmul(
                                ps[:],
                                xT[:, kt, t * P:(t + 1) * P],
                                svsT[:, kt, j * NCH:(j + 1) * NCH],
                                start=(kt == 0),
                                stop=(kt == KS - 1),
                            )
                        uu = wp.tile([P, NCH], f32, tag="uu")
                        nc.vector.tensor_tensor(
                            uu[:], ps[:], s2b[:, j * NCH:(j + 1) * NCH],
                            ALU.subtract,
                        )
                        ee = wp.tile([P, NCH], f32, tag="ee")
                        nc.scalar.activation(
                            ee[:], uu[:], AF.Exp,
                            bias=x2sc[:, t:t + 1], scale=scale_bc[:],
                            accum_out=parts[t][:, j:j + 1],
                        )
                for t in range(MT):
                    S_t = sp.tile([P, 1], f32, tag="S")
                    nc.vector.reduce_sum(S_t[:], parts[t][:], axis=AX.X)
                    lp = sp.tile([P, 1], f32, tag="lp")
                    nc.scalar.activation(lp[:], S_t[:], AF.Ln)
                    nc.vector.tensor_tensor(lp[:], lp[:], C_bc[:], ALU.add)
                    nc.sync.dma_start(outd[t * P:(t + 1) * P, :], lp[:])

    nc.finalize()
    return nc


def kernel(X: np.ndarray, svs: np.ndarray, scale: np.ndarray) -> np.ndarray:
    from concourse.bass_utils import run_bass_kernel_spmd

    if "nc" not in _CACHE:
        _CACHE["nc"] = _build_nc()
    nc = _CACHE["nc"]

    X = np.ascontiguousarray(X, dtype=np.float32)
    svs = np.ascontiguousarray(svs, dtype=np.float32)
    sc = np.asarray(scale, dtype=np.float32).reshape(1)

    in_maps = [
        {"X": X[i * M_LOC:(i + 1) * M_LOC], "svs": svs, "scale": sc}
        for i in range(NCORES)
    ]
    res = run_bass_kernel_spmd(nc, in_maps, core_ids=list(range(NCORES)))
    out = np.concatenate([r["out"].reshape(M_LOC) for r in res.results])
    return out.astype(np.float32)


# revision 9
# speedup vs baseline: 14690.9135x; 14690.9135x over previous
"""KDE log-density kernel for Trainium2, SPMD across 8 NeuronCores.

Computes log_p[m] = logsumexp_n(-scale * ||X[m] - svs[n]||^2) - log(N)
                    + (D/2) * log(scale/pi)

Sharding: X rows split across 8 cores (1024 rows each); svs + scale
replicated. logsumexp over N is local to each row, no collectives.

Per-core algorithm:
  - Pass over svs: s2[n] = sum_d svs[n,d]^2 (ACT Square+accum), cast
    2*svs to bf16 and bounce through DRAM for a DMA transpose into
    svsT [128(d), 4, 8192(n)] resident in SBUF.
  - Same for X: x2, bf16 X^T tiles.
  - s2 broadcast to all 128 partitions via identity-transpose + rank-1
    matmuls -> s2b [128, 8192] f32.
  - Main loop over 8 m-tiles x 16 n-chunks: 4 matmuls (K=128 each)
    accumulate psum = 2*X.svs; DVE tensor_tensor_reduce computes
    u = psum - s2[n] in-place and max_n(u) in one op; online-max
    rescale (flash style); ACT computes exp(scale*u + bias) in-place
    with a fused row-sum accum_out.
  - log_p = ln(acc) + M + C,  C = -ln(N) + (D/2)*(ln(scale) - ln(pi)).
"""

import sys

for _p in ("/opt/trn_rl_repo", "/opt/pypackages"):
    if _p not in sys.path:
        sys.path.insert(0, _p)

import numpy as np

M_FULL, N, D = 8192, 8192, 512
NCORES = 8
M_LOC = M_FULL // NCORES  # 1024
P = 128
KS = D // P               # 4 k-subtiles
MT = M_LOC // P           # 8 m-tiles per core
NCH = 512                 # n-chunk (one PSUM bank of f32)
NCHUNKS = N // NCH        # 16

_CACHE = {}
USE_ONLINE_MAX = False


def _build_nc():
    import concourse.mybir as mybir
    import concourse.tile as tile
    from concourse import bacc

    f32 = mybir.dt.float32
    bf16 = mybir.dt.bfloat16
    AF = mybir.ActivationFunctionType
    ALU = mybir.AluOpType
    AX = mybir.AxisListType

    nc = bacc.Bacc(None, target_bir_lowering=False, debug=True)
    Xd = nc.declare_dram_parameter("X", [M_LOC, D], f32, isOutput=False)
    Sd = nc.declare_dram_parameter("svs", [N, D], f32, isOutput=False)
    scd = nc.declare_dram_parameter("scale", [1], f32, isOutput=False)
    outd = nc.declare_dram_parameter("out", [M_LOC, 1], f32, isOutput=True)

    LOG_CONST = float(-np.log(N) - (D / 2) * np.log(np.pi))

    with tile.TileContext(nc) as tc:
        with (
            tc.tile_pool(name="const", bufs=1) as cp,
            tc.tile_pool(name="work", bufs=3) as wp,
            tc.tile_pool(name="small", bufs=4) as sp,
            tc.tile_pool(name="mmpsum", bufs=6, space="PSUM") as pp,
            tc.tile_pool(name="bcpsum", bufs=2, space="PSUM") as pb,
            tc.tile_pool(name="dram", bufs=1, space="DRAM") as dp,
        ):
            # ---------------- constants ----------------
            scale_bc = cp.tile([P, 1], f32)
            nc.sync.dma_start(scale_bc[:], scd[None, :].to_broadcast((P, 1)))
            negscale = cp.tile([P, 1], f32)
            nc.scalar.mul(negscale[:], scale_bc[:], -1.0)
            # C = (D/2)*ln(scale) + (-ln(N) - (D/2)*ln(pi))
            C_bc = cp.tile([P, 1], f32)
            nc.scalar.activation(C_bc[:], scale_bc[:], AF.Ln)
            nc.vector.tensor_scalar(
                C_bc[:], C_bc[:], float(D / 2), LOG_CONST, ALU.mult, ALU.add
            )

            # ---------------- svs pass: s2 + bf16(2*svs) ----------------
            # chunk c holds svs rows {p*64 + c}: s2_all[p, c] = s2(p*64+c)
            # flattens p-major directly into natural n order.
            NC64 = N // P  # 64 chunks
            s2_all = cp.tile([P, NC64], f32)
            svs2b = dp.tile([N, D], bf16)
            for c in range(NC64):
                sv = wp.tile([P, D], f32, tag="in_f32")
                nc.sync.dma_start(sv[:], Sd[c::NC64, :])
                sq = wp.tile([P, D], f32, tag="sq")
                nc.scalar.activation(
                    sq[:], sv[:], AF.Square, accum_out=s2_all[:, c:c + 1]
                )
                svb = wp.tile([P, D], bf16, tag="out_b16")
                nc.vector.tensor_scalar_mul(svb[:], sv[:], 2.0)
                nc.sync.dma_start(svs2b[c::NC64, :], svb[:])

            s2_dram = dp.tile([N], f32)
            nc.sync.dma_start(s2_dram.rearrange("(p c) -> p c", p=P), s2_all[:])
            # broadcast s2 to all partitions via stride-0 DMA
            s2b = cp.tile([P, N], f32)
            nc.sync.dma_start(s2b[:], s2_dram[None, :].to_broadcast((P, N)))

            # ---------------- X pass: x2 + bf16(X) ----------------
            x2sc = cp.tile([P, MT], f32)  # -scale * x2, per m-tile column
            x2_all = cp.tile([P, MT], f32)
            xb_dram = dp.tile([M_LOC, D], bf16)
            for t in range(MT):
                xv = wp.tile([P, D], f32, tag="in_f32")
                nc.sync.dma_start(xv[:], Xd[t * P:(t + 1) * P, :])
                xsq = wp.tile([P, D], f32, tag="sq")
                nc.scalar.activation(
                    xsq[:], xv[:], AF.Square, accum_out=x2_all[:, t:t + 1]
                )
                xbv = wp.tile([P, D], bf16, tag="out_b16")
                nc.vector.tensor_copy(xbv[:], xv[:])
                nc.sync.dma_start(xb_dram[t * P:(t + 1) * P, :], xbv[:])
                nc.vector.tensor_tensor(
                    x2sc[:, t:t + 1], x2_all[:, t:t + 1], negscale[:], ALU.mult
                )

            # ---------------- DMA transposes (bf16) ----------------
            svsT = cp.tile([P, KS, N], bf16)   # [d%128, d//128, n]
            nc.sync.dma_start_transpose(svsT[:], svs2b[:])
            xT = cp.tile([P, KS, M_LOC], bf16)
            nc.sync.dma_start_transpose(xT[:], xb_dram[:])

            # ---------------- main loop ----------------
            if USE_ONLINE_MAX:
                for t in range(MT):
                    B_t = sp.tile([P, 1], f32, tag=f"B{t}")
                    acc_t = sp.tile([P, 1], f32, tag=f"acc{t}")
                    for j in range(NCHUNKS):
                        ps = pp.tile([P, NCH], f32, tag="mm")
                        for kt in range(KS):
                            nc.tensor.matmul(
                                ps[:],
                                xT[:, kt, t * P:(t + 1) * P],
                                svsT[:, kt, j * NCH:(j + 1) * NCH],
                                start=(kt == 0),
                                stop=(kt == KS - 1),
                            )
                        mxu = sp.tile([P, 1], f32, tag="mxu")
                        uu = wp.tile([P, NCH], f32, tag="uu")
                        nc.vector.tensor_tensor_reduce(
                            out=uu[:],
                            in0=ps[:],
                            in1=s2b[:, j * NCH:(j + 1) * NCH],
                            scale=1.0,
                            scalar=-3.4e38,
                            op0=ALU.subtract,
                            op1=ALU.max,
                            accum_out=mxu[:],
                        )
                        nm = sp.tile([P, 1], f32, tag="nm")
                        nc.vector.tensor_tensor(nm[:], mxu[:], negscale[:], ALU.mult)
                        pj = sp.tile([P, 1], f32, tag="pj")
                        ee = wp.tile([P, NCH], f32, tag="ee")
                        if j == 0:
                            nc.vector.tensor_copy(B_t[:], nm[:])
                            nc.scalar.activation(
                                ee[:], uu[:], AF.Exp,
                                bias=B_t[:], scale=scale_bc[:], accum_out=pj[:],
                            )
                            nc.vector.tensor_copy(acc_t[:], pj[:])
                        else:
                            dd = sp.tile([P, 1], f32, tag="dd")
                            nc.vector.tensor_scalar(
                                dd[:], nm[:], B_t[:], 0.0, ALU.subtract, ALU.min
                            )
                            nc.vector.tensor_tensor(B_t[:], B_t[:], dd[:], ALU.add)
                            rr = sp.tile([P, 1], f32, tag="rr")
                            nc.scalar.activation(rr[:], dd[:], AF.Exp)
                            nc.scalar.activation(
                                ee[:], uu[:], AF.Exp,
                                bias=B_t[:], scale=scale_bc[:], accum_out=pj[:],
                            )
                            nc.vector.tensor_scalar(
                                acc_t[:], acc_t[:], rr[:], pj[:], ALU.mult, ALU.add
                            )
                    mfin = sp.tile([P, 1], f32, tag="mfin")
                    nc.vector.tensor_tensor(
                        mfin[:], x2sc[:, t:t + 1], B_t[:], ALU.subtract
                    )
                    nc.vector.tensor_tensor(mfin[:], mfin[:], C_bc[:], ALU.add)
                    lp = sp.tile([P, 1], f32, tag="lp")
                    nc.scalar.activation(lp[:], acc_t[:], AF.Ln)
                    nc.vector.tensor_tensor(lp[:], lp[:], mfin[:], ALU.add)
                    nc.sync.dma_start(outd[t * P:(t + 1) * P, :], lp[:])
            else:
                for t in range(MT):
                    partials = sp.tile([P, NCHUNKS], f32, tag=f"part{t}")
                    for j in range(NCHUNKS):
                        ps = pp.tile([P, NCH], f32, tag="mm")
                        for kt in range(KS):
                            nc.tensor.matmul(
                                ps[:],
                                xT[:, kt, t * P:(t + 1) * P],
                                svsT[:, kt, j * NCH:(j + 1) * NCH],
                                start=(kt == 0),
                                stop=(kt == KS - 1),
                            )
                        uu = wp.tile([P, NCH], f32, tag="uu")
                        nc.vector.tensor_tensor(
                            uu[:], ps[:], s2b[:, j * NCH:(j + 1) * NCH],
                            ALU.subtract,
                        )
                        ee = wp.tile([P, NCH], f32, tag="ee")
                        nc.scalar.activation(
                            ee[:], uu[:], AF.Exp,
                            bias=x2sc[:, t:t + 1], scale=scale_bc[:],
                            accum_out=partials[:, j:j + 1],
                        )
                    S_t = sp.tile([P, 1], f32, tag="S")
                    nc.vector.reduce_sum(S_t[:], partials[:], axis=AX.X)
                    lp = sp.tile([P, 1], f32, tag="lp")
                    nc.scalar.activation(lp[:], S_t[:], AF.Ln)
                    nc.vector.tensor_tensor(lp[:], lp[:], C_bc[:], ALU.add)
                    nc.sync.dma_start(outd[t * P:(t + 1) * P, :], lp[:])

    nc.finalize()
    return nc


def kernel(X: np.ndarray, svs: np.ndarray, scale: np.ndarray) -> np.ndarray:
    from concourse.bass_utils import run_bass_kernel_spmd

    if "nc" not in _CACHE:
        _CACHE["nc"] = _build_nc()
    nc = _CACHE["nc"]

    X = np.ascontiguousarray(X, dtype=np.float32)
    svs = np.ascontiguousarray(svs, dtype=np.float32)
    sc = np.asarray(scale, dtype=np.float32).reshape(1)

    in_maps = [
        {"X": X[i * M_LOC:(i + 1) * M_LOC], "svs": svs, "scale": sc}
        for i in range(NCORES)
    ]
    res = run_bass_kernel_spmd(nc, in_maps, core_ids=list(range(NCORES)))
    out = np.concatenate([r["out"].reshape(M_LOC) for r in res.results])
    return out.astype(np.float32)
